# revision 36
# baseline (speedup 1.0000x reference)
"""Trainium2 Bass kernel for the BayesianFilter (racing-line posterior) problem.

Math (per sample s, P=256 curve points, n=7 Bezier order):
    curves = curve + noise[s]                       # [8,2]
    v  = (M_D1 @ (n*D1) @ curves) / dT              # [P,2]
    a  = (M_D2 @ (n*(n-1)*D2) @ curves) / dT^2      # [P,2]
    speed = |v|, lin = (a.v)/speed
    blim = interp(speed, xp, fp)   (piecewise linear, clamped)
    viol = min(lin - blim, 0);  brake = exp(mean_p viol)
    ca_score = clip(exp(relu(...)), 0, 1) == 1.0 identically  -> dropped
    sp = brake;  out = sum_s (sp/sum sp) * curves[s]

Device computes red[s] = sum_p relu(blim - lin) for all samples
(data-parallel over 8 cores, 8192 samples each); the exp, normalization and
the tiny weighted [8,2] sum run on host.

Device layout: partitions = 128 curve points (2 halves), free = samples.
    v/a via PE matmuls (bf16 in, f32 PSUM out):
        out[p, s] = B[9,128].T @ noise_aug[9, s]
    (noise_aug rows = 8 transposed noise components + ones row; B rows =
    folded coefficient matrix + bias column from `curve`).

Engine budget per half-block (x|y pairs packed in [128,1024] PSUM tiles;
cost-model busy per core: ACT 98us, DVE 95us, POOL 72us, PE 35us, end 122us):
    PE  : 4 matmuls (bf16) into vxy/axy pairs + 2 column-sum matmuls
    ACT : copy(axy) [PSUM->SBUF bf16], square(vxy) -> bf16, sqrt
          (one activation table set -> a single table load)
    DVE : dprod = caxy*vxy (1x, PSUM operand), s2 add (bf16 2x),
          recip_approx(speed), bclip = b*min(speed,xmax) (2x),
          u = bclip-lin (bf16 2x), relu+bias (bf16 4x)
    POOL: dot = dx+dy, lin = dot*rs
"""

import numpy as np
import ml_dtypes
from math import comb

# ---------------------------------------------------------------- constants
NUM_POINTS = 256
ORDER = 7
NUM_SAMPLES = 65536
N_CORES = 8
BETA_BRAKE = 1.0
S_CORE = NUM_SAMPLES // N_CORES          # 8192 samples per core
NBLK = 16                                # sample blocks per core
BLK = S_CORE // NBLK                     # 512 samples per block
HALF = 128                               # points per partition-tile

_PROGRAM_CACHE: dict = {}
LAST_RESULTS = None


def _bezier_matrix(num_points, order):
    s = np.linspace(0.0, 1.0, num_points)[:, None]
    k = np.arange(order + 1)[None, :]
    binom = np.array([comb(order, i) for i in range(order + 1)], dtype=np.float64)[None, :]
    return (binom * (s ** k) * ((1.0 - s) ** (order - k))).astype(np.float32)


def _coeff_matrices(deltaT):
    """A1 [256,8] and A2 [256,8]: point-velocity / acceleration as linear maps
    of the 8 control points (per spatial dim)."""
    n = ORDER
    M1 = _bezier_matrix(NUM_POINTS, n - 1).astype(np.float64)   # [P, 7]
    M2 = _bezier_matrix(NUM_POINTS, n - 2).astype(np.float64)   # [P, 6]
    D1 = np.zeros((n, n + 1))
    for j in range(n):
        D1[j, j] = -1.0
        D1[j, j + 1] = 1.0
    D2 = np.zeros((n - 1, n + 1))
    for j in range(n - 1):
        D2[j, j] = 1.0
        D2[j, j + 1] = -2.0
        D2[j, j + 2] = 1.0
    A1 = (M1 @ (n * D1)) / float(deltaT)
    A2 = (M2 @ (n * (n - 1) * D2)) / (float(deltaT) ** 2)
    return A1.astype(np.float32), A2.astype(np.float32)


def _interp_params(xp, fp):
    """If the table is a strictly-increasing, globally-linear ramp return
    (a, b) with f(x) = a + b*clip(x, xp[0], xp[-1]); else None."""
    xp = np.asarray(xp, np.float64)
    fp = np.asarray(fp, np.float64)
    dx = np.diff(xp)
    if not (dx > 0).all():
        return None
    slopes = np.diff(fp) / dx
    b = slopes[0]
    if not np.allclose(slopes, b, rtol=1e-5, atol=1e-7):
        return None
    a = fp[0] - b * xp[0]
    return float(a), float(b)


# ------------------------------------------------------------ device program
def _build_program(a, b, xmin, xmax, generic_knots=None):
    """Trace + compile the single-core SPMD program.

    Inputs (per core): bmats [9, 1024] bf16, nx [9, 8192] bf16, ny [9, 8192] bf16.
    Output: red [16, 512] f32 — per-sample sum_p relu(blim - lin)
    (host computes sp = exp(-BETA/P * red)).

    generic_knots: None for the linear-interp fast path, else a tuple
    (xp list[16], d list[15], y0) for the relu-sum piecewise path.
    """
    import concourse.bacc as bacc
    import concourse.tile as tile
    import concourse.mybir as mybir

    f32 = mybir.dt.float32
    bf16 = mybir.dt.bfloat16
    Act = mybir.ActivationFunctionType
    Alu = mybir.AluOpType

    nc = bacc.Bacc("TRN2", target_bir_lowering=False, debug=False)

    bmats_d = nc.dram_tensor("bmats", [9, 8 * HALF], bf16, kind="ExternalInput").ap()
    nx_d = nc.dram_tensor("nx", [9, S_CORE], bf16, kind="ExternalInput").ap()
    ny_d = nc.dram_tensor("ny", [9, S_CORE], bf16, kind="ExternalInput").ap()
    # NOTE: 1-D ExternalOutput tensors fail at NEFF LoadExecutable under the
    # axon/PJRT path — keep DRAM I/O 2-D.
    red_d = nc.dram_tensor("red", [NBLK, BLK], f32, kind="ExternalOutput").ap()

    with tile.TileContext(nc) as tc:
        with (
            tc.tile_pool(name="const", bufs=1) as const_pool,
            tc.tile_pool(name="rhs", bufs=4) as rhs_pool,
            tc.tile_pool(name="work", bufs=6) as work,
            tc.tile_pool(name="spout", bufs=4) as spout_pool,
            tc.tile_pool(name="mmv", bufs=2, space="PSUM") as mmv_pool,
            tc.tile_pool(name="mma", bufs=1, space="PSUM") as mma_pool,
            tc.tile_pool(name="red", bufs=1, space="PSUM") as red_pool,
        ):
            bm = const_pool.tile([9, 8 * HALF], bf16, tag="bm")
            nc.sync.dma_start(bm[:], bmats_d)
            ones = const_pool.tile([HALF, 1], bf16, tag="ones")
            nc.vector.memset(ones[:], 1.0)
            bias_knots = []
            if generic_knots is not None:
                for i, xk in enumerate(generic_knots[0]):
                    t = const_pool.tile([HALF, 1], f32, tag=f"bias_k{i}")
                    nc.vector.memset(t[:], -float(xk))
                    bias_knots.append(t)

            # lhsT blocks in bmats: [vx_h0, vx_h1, vy_h0, vy_h1, ax_h0, ax_h1, ay_h0, ay_h1]
            def bmat(i):
                return bm[:, i * HALF:(i + 1) * HALF]

            pending_out = []

            def flush_out():
                while pending_out:
                    kk, t = pending_out.pop(0)
                    nc.sync.dma_start(red_d[kk:kk + 1, :], t[:])

            rxp = ryp = None
            for k in range(NBLK):
                if k % 2 == 0:
                    rxp = rhs_pool.tile([9, 2 * BLK], bf16, tag="rx")
                    nc.sync.dma_start(rxp[:], nx_d[:, k * BLK:(k + 2) * BLK])
                    ryp = rhs_pool.tile([9, 2 * BLK], bf16, tag="ry")
                    nc.sync.dma_start(ryp[:], ny_d[:, k * BLK:(k + 2) * BLK])
                ks = slice((k % 2) * BLK, (k % 2 + 1) * BLK)
                rx = rxp[:, ks]
                ry = ryp[:, ks]
                flush_out()

                red = red_pool.tile([1, BLK], f32, tag="red")
                for h in range(2):
                    # x|y paired PSUM tiles: one 2-bank tile per (v, a)
                    vxy = mmv_pool.tile([HALF, 2 * BLK], f32, tag="vxy")
                    nc.tensor.matmul(vxy[:, 0:BLK], bmat(0 + h), rx[:],
                                     start=True, stop=True)
                    nc.tensor.matmul(vxy[:, BLK:2 * BLK], bmat(2 + h), ry[:],
                                     start=True, stop=True)
                    axy = mma_pool.tile([HALF, 2 * BLK], f32, tag="axy")
                    nc.tensor.matmul(axy[:, 0:BLK], bmat(4 + h), rx[:],
                                     start=True, stop=True)
                    nc.tensor.matmul(axy[:, BLK:2 * BLK], bmat(6 + h), ry[:],
                                     start=True, stop=True)

                    # ACT: one copy + one square over the x|y pair, one sqrt
                    caxy = work.tile([HALF, 2 * BLK], bf16, tag="caxy")
                    nc.scalar.copy(caxy[:], axy[:])
                    sqp = work.tile([HALF, 2 * BLK], bf16, tag="sqp")
                    nc.scalar.square(sqp[:], vxy[:])
                    # DVE: dprod = a*v for x|y in one pass (PSUM operand, 1x)
                    dprod = work.tile([HALF, 2 * BLK], bf16, tag="dprod")
                    nc.vector.tensor_mul(dprod[:], caxy[:], vxy[:])
                    # DVE bf16 2x: s2 = vx^2 + vy^2
                    s2 = work.tile([HALF, BLK], bf16, tag="s2")
                    nc.vector.tensor_add(s2[:], sqp[:, 0:BLK], sqp[:, BLK:2 * BLK])
                    speed = work.tile([HALF, BLK], f32, tag="speed")
                    nc.scalar.sqrt(speed[:], s2[:])
                    rs = work.tile([HALF, BLK], f32, tag="rs")
                    nc.vector.reciprocal_approx_fast(out=rs[:], in_=speed[:])
                    # POOL: dot = dx + dy, lin = dot * rs
                    dot = work.tile([HALF, BLK], bf16, tag="dot")
                    nc.gpsimd.tensor_add(dot[:], dprod[:, 0:BLK],
                                         dprod[:, BLK:2 * BLK])
                    lin = work.tile([HALF, BLK], bf16, tag="lin")
                    nc.gpsimd.tensor_mul(lin[:], dot[:], rs[:])

                    ru = work.tile([HALF, BLK], bf16, tag="ru")
                    if generic_knots is None and xmin <= 0.0:
                        # speed >= 0 >= xmin, so clip lower bound is a no-op:
                        # bclip = b*min(speed, xmax)  (one 2x tensor_scalar)
                        bclip = work.tile([HALF, BLK], bf16, tag="bclip")
                        nc.vector.tensor_scalar(
                            out=bclip[:], in0=speed[:],
                            scalar1=float(xmax), scalar2=float(b),
                            op0=Alu.min, op1=Alu.mult,
                        )
                        u = work.tile([HALF, BLK], bf16, tag="u")
                        nc.vector.tensor_sub(u[:], bclip[:], lin[:])
                        # ru = relu(u + a) = relu(blim - lin) = -viol   (bf16 4x)
                        nc.vector.tensor_scalar(
                            out=ru[:], in0=u[:],
                            scalar1=float(a), scalar2=0.0,
                            op0=Alu.add, op1=Alu.max,
                        )
                    elif generic_knots is None:
                        # u = b*clip(speed, xmin, xmax) - lin
                        clipv = work.tile([HALF, BLK], bf16, tag="clipv")
                        nc.vector.tensor_scalar(
                            out=clipv[:], in0=speed[:],
                            scalar1=float(xmin), scalar2=float(xmax),
                            op0=Alu.max, op1=Alu.min,
                        )
                        u = work.tile([HALF, BLK], bf16, tag="u")
                        nc.vector.scalar_tensor_tensor(
                            out=u[:], in0=clipv[:], scalar=float(b), in1=lin[:],
                            op0=Alu.mult, op1=Alu.subtract,
                        )
                        nc.vector.tensor_scalar(
                            out=ru[:], in0=u[:],
                            scalar1=float(a), scalar2=0.0,
                            op0=Alu.add, op1=Alu.max,
                        )
                    else:
                        xp_k, d_k, y0 = generic_knots
                        clipv = work.tile([HALF, BLK], f32, tag="clipv")
                        nc.vector.tensor_scalar(
                            out=clipv[:], in0=speed[:],
                            scalar1=float(xp_k[0]), scalar2=float(xp_k[-1]),
                            op0=Alu.max, op1=Alu.min,
                        )
                        # blim(x) = y0 + sum_i d_i * relu(x - xp_i)
                        acc = work.tile([HALF, BLK], f32, tag="acc")
                        ri = work.tile([HALF, BLK], f32, tag="ri")
                        nc.scalar.activation(ri[:], clipv[:], Act.Relu,
                                             bias=bias_knots[0][:])
                        nc.vector.tensor_scalar(
                            out=acc[:], in0=ri[:],
                            scalar1=float(d_k[0]), scalar2=float(y0),
                            op0=Alu.mult, op1=Alu.add,
                        )
                        for i in range(1, len(d_k)):
                            ri = work.tile([HALF, BLK], f32, tag="ri")
                            nc.scalar.activation(ri[:], clipv[:], Act.Relu,
                                                 bias=bias_knots[i][:])
                            nc.vector.scalar_tensor_tensor(
                                out=acc[:], in0=ri[:], scalar=float(d_k[i]),
                                in1=acc[:], op0=Alu.mult, op1=Alu.add,
                            )
                        u = work.tile([HALF, BLK], f32, tag="u")
                        nc.vector.tensor_sub(u[:], acc[:], lin[:])
                        nc.vector.tensor_scalar(
                            out=ru[:], in0=u[:], scalar1=0.0, scalar2=None,
                            op0=Alu.max,
                        )

                    # red[0, s] += sum_p ru[p, s]   (PE column-sum, bf16 in f32 acc)
                    nc.tensor.matmul(red[:], ones[:], ru[:],
                                     start=(h == 0), stop=(h == 1))

                out_t = spout_pool.tile([1, BLK], f32, tag="out")
                nc.scalar.copy(out_t[:], red[:])
                pending_out.append((k, out_t))
            flush_out()

    nc.compile()
    return nc


def _get_program(key_params, generic_knots=None):
    key = (key_params, None if generic_knots is None else
           (tuple(generic_knots[0]), tuple(generic_knots[1]), generic_knots[2]))
    prog = _PROGRAM_CACHE.get(key)
    if prog is None:
        a, b, xmin, xmax = key_params
        prog = _build_program(a, b, xmin, xmax, generic_knots)
        _PROGRAM_CACHE[key] = prog
    return prog


def _core_inputs(noise, bmats_bf):
    """Per-core input dicts: transposed bf16 noise components + ones row."""
    ins = []
    for cidx in range(N_CORES):
        sl = noise[cidx * S_CORE:(cidx + 1) * S_CORE]        # [8192, 8, 2]
        nxa = np.empty((9, S_CORE), ml_dtypes.bfloat16)
        nxa[:8] = sl[:, :, 0].T.astype(ml_dtypes.bfloat16)
        nxa[8] = 1.0
        nya = np.empty((9, S_CORE), ml_dtypes.bfloat16)
        nya[:8] = sl[:, :, 1].T.astype(ml_dtypes.bfloat16)
        nya[8] = 1.0
        ins.append({"bmats": bmats_bf, "nx": np.ascontiguousarray(nxa),
                    "ny": np.ascontiguousarray(nya)})
    return ins


def _build_bmats(A1, A2, c1, c2):
    # bmats: 8 blocks [9, 128]: rows 0-7 = A.T half, row 8 = bias column
    # order: vx_h0, vx_h1, vy_h0, vy_h1, ax_h0, ax_h1, ay_h0, ay_h1
    blocks = []
    for (A, c) in ((A1, c1), (A2, c2)):
        for d_ in range(2):
            for h in range(2):
                blk = np.empty((9, HALF), np.float32)
                blk[:8] = A[h * HALF:(h + 1) * HALF, :].T
                blk[8] = c[h * HALF:(h + 1) * HALF, d_]
                blocks.append(blk)
    bmats = np.concatenate(blocks, axis=1)                    # [9, 1024]
    return np.ascontiguousarray(bmats.astype(ml_dtypes.bfloat16))


# ------------------------------------------------------------------- kernel
def kernel(curve, noise, speeds_table, braking_limits_table, deltaT):
    curve = np.asarray(curve, np.float32)
    noise = np.asarray(noise, np.float32)
    xp = np.asarray(speeds_table, np.float32)
    fp = np.asarray(braking_limits_table, np.float32)
    dT = float(np.asarray(deltaT))

    A1, A2 = _coeff_matrices(dT)                    # [256, 8] each
    c1 = A1 @ curve                                 # [256, 2]
    c2 = A2 @ curve

    lin_ab = _interp_params(xp, fp)
    if lin_ab is not None:
        a, b = lin_ab
        generic = None
    else:
        xpd = xp.astype(np.float64)
        fpd = fp.astype(np.float64)
        slopes = np.diff(fpd) / np.diff(xpd)
        d = np.concatenate([[slopes[0]], np.diff(slopes)])
        generic = (list(map(float, xpd[:-1])), list(map(float, d)), float(fpd[0]))
        a, b = 0.0, 0.0
    xmin, xmax = float(xp[0]), float(xp[-1])

    bmats = _build_bmats(A1, A2, c1, c2)
    prog = _get_program((a, b, xmin, xmax), generic)
    in_maps = _core_inputs(noise, bmats)

    from concourse.bass_utils import run_bass_kernel_spmd
    res = run_bass_kernel_spmd(prog, in_maps, list(range(N_CORES)))
    global LAST_RESULTS
    LAST_RESULTS = res
    red = np.concatenate([res.results[i]["red"].reshape(-1)
                          for i in range(N_CORES)])

    spd = np.exp(-BETA_BRAKE / NUM_POINTS * red.astype(np.float64))
    probs = spd / spd.sum()
    wsum = probs @ noise.reshape(NUM_SAMPLES, -1).astype(np.float64)
    out = curve.astype(np.float64) + wsum.reshape(ORDER + 1, 2)
    return out.astype(np.float32)


# revision 37
# speedup vs baseline: 1.0010x; 1.0010x over previous
"""Trainium2 Bass kernel for the BayesianFilter (racing-line posterior) problem.

Math (per sample s, P=256 curve points, n=7 Bezier order):
    curves = curve + noise[s]                       # [8,2]
    v  = (M_D1 @ (n*D1) @ curves) / dT              # [P,2]
    a  = (M_D2 @ (n*(n-1)*D2) @ curves) / dT^2      # [P,2]
    speed = |v|, lin = (a.v)/speed
    blim = interp(speed, xp, fp)   (piecewise linear, clamped)
    viol = min(lin - blim, 0);  brake = exp(mean_p viol)
    ca_score = clip(exp(relu(...)), 0, 1) == 1.0 identically  -> dropped
    sp = brake;  out = sum_s (sp/sum sp) * curves[s]

Device computes red[s] = sum_p relu(blim - lin) for all samples
(data-parallel over 8 cores, 8192 samples each); the exp, normalization and
the tiny weighted [8,2] sum run on host.

Device layout: partitions = 128 curve points (2 halves), free = samples.
    v/a via PE matmuls (bf16 in, f32 PSUM out):
        out[p, s] = B[9,128].T @ noise_aug[9, s]
    (noise_aug rows = 8 transposed noise components + ones row; B rows =
    folded coefficient matrix + bias column from `curve`).

Engine budget per half-block (x|y pairs packed in [128,1024] PSUM tiles;
cost-model busy per core: ACT 98us, DVE 95us, POOL 72us, PE 35us, end 122us):
    PE  : 4 matmuls (bf16) into vxy/axy pairs + 2 column-sum matmuls
    ACT : copy(axy) [PSUM->SBUF bf16], square(vxy) -> bf16, sqrt
          (one activation table set -> a single table load)
    DVE : dprod = caxy*vxy (1x, PSUM operand), s2 add (bf16 2x),
          recip_approx(speed), bclip = b*min(speed,xmax) (2x),
          u = bclip-lin (bf16 2x), relu+bias (bf16 4x)
    POOL: dot = dx+dy, lin = dot*rs
"""

import numpy as np
import ml_dtypes
from math import comb

# ---------------------------------------------------------------- constants
NUM_POINTS = 256
ORDER = 7
NUM_SAMPLES = 65536
N_CORES = 8
BETA_BRAKE = 1.0
S_CORE = NUM_SAMPLES // N_CORES          # 8192 samples per core
NBLK = 16                                # sample blocks per core
BLK = S_CORE // NBLK                     # 512 samples per block
HALF = 128                               # points per partition-tile

_PROGRAM_CACHE: dict = {}
LAST_RESULTS = None


def _bezier_matrix(num_points, order):
    s = np.linspace(0.0, 1.0, num_points)[:, None]
    k = np.arange(order + 1)[None, :]
    binom = np.array([comb(order, i) for i in range(order + 1)], dtype=np.float64)[None, :]
    return (binom * (s ** k) * ((1.0 - s) ** (order - k))).astype(np.float32)


def _coeff_matrices(deltaT):
    """A1 [256,8] and A2 [256,8]: point-velocity / acceleration as linear maps
    of the 8 control points (per spatial dim)."""
    n = ORDER
    M1 = _bezier_matrix(NUM_POINTS, n - 1).astype(np.float64)   # [P, 7]
    M2 = _bezier_matrix(NUM_POINTS, n - 2).astype(np.float64)   # [P, 6]
    D1 = np.zeros((n, n + 1))
    for j in range(n):
        D1[j, j] = -1.0
        D1[j, j + 1] = 1.0
    D2 = np.zeros((n - 1, n + 1))
    for j in range(n - 1):
        D2[j, j] = 1.0
        D2[j, j + 1] = -2.0
        D2[j, j + 2] = 1.0
    A1 = (M1 @ (n * D1)) / float(deltaT)
    A2 = (M2 @ (n * (n - 1) * D2)) / (float(deltaT) ** 2)
    return A1.astype(np.float32), A2.astype(np.float32)


def _interp_params(xp, fp):
    """If the table is a strictly-increasing, globally-linear ramp return
    (a, b) with f(x) = a + b*clip(x, xp[0], xp[-1]); else None."""
    xp = np.asarray(xp, np.float64)
    fp = np.asarray(fp, np.float64)
    dx = np.diff(xp)
    if not (dx > 0).all():
        return None
    slopes = np.diff(fp) / dx
    b = slopes[0]
    if not np.allclose(slopes, b, rtol=1e-5, atol=1e-7):
        return None
    a = fp[0] - b * xp[0]
    return float(a), float(b)


# ------------------------------------------------------------ device program
def _build_program(a, b, xmin, xmax, generic_knots=None):
    """Trace + compile the single-core SPMD program.

    Inputs (per core): bmats [9, 1024] bf16, nx [9, 8192] bf16, ny [9, 8192] bf16.
    Output: red [16, 512] f32 — per-sample sum_p relu(blim - lin)
    (host computes sp = exp(-BETA/P * red)).

    generic_knots: None for the linear-interp fast path, else a tuple
    (xp list[16], d list[15], y0) for the relu-sum piecewise path.
    """
    import concourse.bacc as bacc
    import concourse.tile as tile
    import concourse.mybir as mybir

    f32 = mybir.dt.float32
    bf16 = mybir.dt.bfloat16
    Act = mybir.ActivationFunctionType
    Alu = mybir.AluOpType

    nc = bacc.Bacc("TRN2", target_bir_lowering=False, debug=False)

    bmats_d = nc.dram_tensor("bmats", [9, 8 * HALF], bf16, kind="ExternalInput").ap()
    nx_d = nc.dram_tensor("nx", [9, S_CORE], bf16, kind="ExternalInput").ap()
    ny_d = nc.dram_tensor("ny", [9, S_CORE], bf16, kind="ExternalInput").ap()
    # NOTE: 1-D ExternalOutput tensors fail at NEFF LoadExecutable under the
    # axon/PJRT path — keep DRAM I/O 2-D.
    red_d = nc.dram_tensor("red", [NBLK, BLK], f32, kind="ExternalOutput").ap()

    with tile.TileContext(nc) as tc:
        with (
            tc.tile_pool(name="const", bufs=1) as const_pool,
            tc.tile_pool(name="rhs", bufs=4) as rhs_pool,
            tc.tile_pool(name="work", bufs=6) as work,
            tc.tile_pool(name="spout", bufs=4) as spout_pool,
            tc.tile_pool(name="mmv", bufs=2, space="PSUM") as mmv_pool,
            tc.tile_pool(name="mma", bufs=1, space="PSUM") as mma_pool,
            tc.tile_pool(name="red", bufs=1, space="PSUM") as red_pool,
        ):
            bm = const_pool.tile([9, 8 * HALF], bf16, tag="bm")
            nc.sync.dma_start(bm[:], bmats_d)
            ones = const_pool.tile([HALF, 1], bf16, tag="ones")
            nc.gpsimd.memset(ones[:], 1.0)
            # pre-warm the sqrt-set activation table while input DMAs run
            warm = const_pool.tile([HALF, 1], f32, tag="warm")
            nc.gpsimd.memset(warm[:], 1.0)
            warm2 = const_pool.tile([HALF, 1], f32, tag="warm2")
            nc.scalar.sqrt(warm2[:], warm[:])
            bias_knots = []
            if generic_knots is not None:
                for i, xk in enumerate(generic_knots[0]):
                    t = const_pool.tile([HALF, 1], f32, tag=f"bias_k{i}")
                    nc.vector.memset(t[:], -float(xk))
                    bias_knots.append(t)

            # lhsT blocks in bmats: [vx_h0, vx_h1, vy_h0, vy_h1, ax_h0, ax_h1, ay_h0, ay_h1]
            def bmat(i):
                return bm[:, i * HALF:(i + 1) * HALF]

            pending_out = []

            def flush_out():
                while pending_out:
                    kk, t = pending_out.pop(0)
                    nc.sync.dma_start(red_d[kk:kk + 1, :], t[:])

            rxp = ryp = None
            for k in range(NBLK):
                if k % 2 == 0:
                    rxp = rhs_pool.tile([9, 2 * BLK], bf16, tag="rx")
                    nc.sync.dma_start(rxp[:], nx_d[:, k * BLK:(k + 2) * BLK])
                    ryp = rhs_pool.tile([9, 2 * BLK], bf16, tag="ry")
                    nc.sync.dma_start(ryp[:], ny_d[:, k * BLK:(k + 2) * BLK])
                ks = slice((k % 2) * BLK, (k % 2 + 1) * BLK)
                rx = rxp[:, ks]
                ry = ryp[:, ks]
                flush_out()

                red = red_pool.tile([1, BLK], f32, tag="red")
                for h in range(2):
                    # x|y paired PSUM tiles: one 2-bank tile per (v, a)
                    vxy = mmv_pool.tile([HALF, 2 * BLK], f32, tag="vxy")
                    nc.tensor.matmul(vxy[:, 0:BLK], bmat(0 + h), rx[:],
                                     start=True, stop=True)
                    nc.tensor.matmul(vxy[:, BLK:2 * BLK], bmat(2 + h), ry[:],
                                     start=True, stop=True)
                    axy = mma_pool.tile([HALF, 2 * BLK], f32, tag="axy")
                    nc.tensor.matmul(axy[:, 0:BLK], bmat(4 + h), rx[:],
                                     start=True, stop=True)
                    nc.tensor.matmul(axy[:, BLK:2 * BLK], bmat(6 + h), ry[:],
                                     start=True, stop=True)

                    # ACT: one copy + one square over the x|y pair, one sqrt
                    caxy = work.tile([HALF, 2 * BLK], bf16, tag="caxy")
                    nc.scalar.copy(caxy[:], axy[:])
                    sqp = work.tile([HALF, 2 * BLK], bf16, tag="sqp")
                    nc.scalar.square(sqp[:], vxy[:])
                    # DVE: dprod = a*v for x|y in one pass (PSUM operand, 1x)
                    dprod = work.tile([HALF, 2 * BLK], bf16, tag="dprod")
                    nc.vector.tensor_mul(dprod[:], caxy[:], vxy[:])
                    # DVE bf16 2x: s2 = vx^2 + vy^2
                    s2 = work.tile([HALF, BLK], bf16, tag="s2")
                    nc.vector.tensor_add(s2[:], sqp[:, 0:BLK], sqp[:, BLK:2 * BLK])
                    speed = work.tile([HALF, BLK], f32, tag="speed")
                    nc.scalar.sqrt(speed[:], s2[:])
                    rs = work.tile([HALF, BLK], f32, tag="rs")
                    nc.vector.reciprocal_approx_fast(out=rs[:], in_=speed[:])
                    # POOL: dot = dx + dy, lin = dot * rs
                    dot = work.tile([HALF, BLK], bf16, tag="dot")
                    nc.gpsimd.tensor_add(dot[:], dprod[:, 0:BLK],
                                         dprod[:, BLK:2 * BLK])
                    lin = work.tile([HALF, BLK], bf16, tag="lin")
                    nc.gpsimd.tensor_mul(lin[:], dot[:], rs[:])

                    ru = work.tile([HALF, BLK], bf16, tag="ru")
                    if generic_knots is None and xmin <= 0.0:
                        # speed >= 0 >= xmin, so clip lower bound is a no-op:
                        # bclip = b*min(speed, xmax)  (one 2x tensor_scalar)
                        bclip = work.tile([HALF, BLK], bf16, tag="bclip")
                        nc.vector.tensor_scalar(
                            out=bclip[:], in0=speed[:],
                            scalar1=float(xmax), scalar2=float(b),
                            op0=Alu.min, op1=Alu.mult,
                        )
                        u = work.tile([HALF, BLK], bf16, tag="u")
                        nc.vector.tensor_sub(u[:], bclip[:], lin[:])
                        # ru = relu(u + a) = relu(blim - lin) = -viol   (bf16 4x)
                        nc.vector.tensor_scalar(
                            out=ru[:], in0=u[:],
                            scalar1=float(a), scalar2=0.0,
                            op0=Alu.add, op1=Alu.max,
                        )
                    elif generic_knots is None:
                        # u = b*clip(speed, xmin, xmax) - lin
                        clipv = work.tile([HALF, BLK], bf16, tag="clipv")
                        nc.vector.tensor_scalar(
                            out=clipv[:], in0=speed[:],
                            scalar1=float(xmin), scalar2=float(xmax),
                            op0=Alu.max, op1=Alu.min,
                        )
                        u = work.tile([HALF, BLK], bf16, tag="u")
                        nc.vector.scalar_tensor_tensor(
                            out=u[:], in0=clipv[:], scalar=float(b), in1=lin[:],
                            op0=Alu.mult, op1=Alu.subtract,
                        )
                        nc.vector.tensor_scalar(
                            out=ru[:], in0=u[:],
                            scalar1=float(a), scalar2=0.0,
                            op0=Alu.add, op1=Alu.max,
                        )
                    else:
                        xp_k, d_k, y0 = generic_knots
                        clipv = work.tile([HALF, BLK], f32, tag="clipv")
                        nc.vector.tensor_scalar(
                            out=clipv[:], in0=speed[:],
                            scalar1=float(xp_k[0]), scalar2=float(xp_k[-1]),
                            op0=Alu.max, op1=Alu.min,
                        )
                        # blim(x) = y0 + sum_i d_i * relu(x - xp_i)
                        acc = work.tile([HALF, BLK], f32, tag="acc")
                        ri = work.tile([HALF, BLK], f32, tag="ri")
                        nc.scalar.activation(ri[:], clipv[:], Act.Relu,
                                             bias=bias_knots[0][:])
                        nc.vector.tensor_scalar(
                            out=acc[:], in0=ri[:],
                            scalar1=float(d_k[0]), scalar2=float(y0),
                            op0=Alu.mult, op1=Alu.add,
                        )
                        for i in range(1, len(d_k)):
                            ri = work.tile([HALF, BLK], f32, tag="ri")
                            nc.scalar.activation(ri[:], clipv[:], Act.Relu,
                                                 bias=bias_knots[i][:])
                            nc.vector.scalar_tensor_tensor(
                                out=acc[:], in0=ri[:], scalar=float(d_k[i]),
                                in1=acc[:], op0=Alu.mult, op1=Alu.add,
                            )
                        u = work.tile([HALF, BLK], f32, tag="u")
                        nc.vector.tensor_sub(u[:], acc[:], lin[:])
                        nc.vector.tensor_scalar(
                            out=ru[:], in0=u[:], scalar1=0.0, scalar2=None,
                            op0=Alu.max,
                        )

                    # red[0, s] += sum_p ru[p, s]   (PE column-sum, bf16 in f32 acc)
                    nc.tensor.matmul(red[:], ones[:], ru[:],
                                     start=(h == 0), stop=(h == 1))

                out_t = spout_pool.tile([1, BLK], f32, tag="out")
                nc.scalar.copy(out_t[:], red[:])
                pending_out.append((k, out_t))
            flush_out()

    nc.compile()
    return nc


def _get_program(key_params, generic_knots=None):
    key = (key_params, None if generic_knots is None else
           (tuple(generic_knots[0]), tuple(generic_knots[1]), generic_knots[2]))
    prog = _PROGRAM_CACHE.get(key)
    if prog is None:
        a, b, xmin, xmax = key_params
        prog = _build_program(a, b, xmin, xmax, generic_knots)
        _PROGRAM_CACHE[key] = prog
    return prog


def _core_inputs(noise, bmats_bf):
    """Per-core input dicts: transposed bf16 noise components + ones row."""
    ins = []
    for cidx in range(N_CORES):
        sl = noise[cidx * S_CORE:(cidx + 1) * S_CORE]        # [8192, 8, 2]
        nxa = np.empty((9, S_CORE), ml_dtypes.bfloat16)
        nxa[:8] = sl[:, :, 0].T.astype(ml_dtypes.bfloat16)
        nxa[8] = 1.0
        nya = np.empty((9, S_CORE), ml_dtypes.bfloat16)
        nya[:8] = sl[:, :, 1].T.astype(ml_dtypes.bfloat16)
        nya[8] = 1.0
        ins.append({"bmats": bmats_bf, "nx": np.ascontiguousarray(nxa),
                    "ny": np.ascontiguousarray(nya)})
    return ins


def _build_bmats(A1, A2, c1, c2):
    # bmats: 8 blocks [9, 128]: rows 0-7 = A.T half, row 8 = bias column
    # order: vx_h0, vx_h1, vy_h0, vy_h1, ax_h0, ax_h1, ay_h0, ay_h1
    blocks = []
    for (A, c) in ((A1, c1), (A2, c2)):
        for d_ in range(2):
            for h in range(2):
                blk = np.empty((9, HALF), np.float32)
                blk[:8] = A[h * HALF:(h + 1) * HALF, :].T
                blk[8] = c[h * HALF:(h + 1) * HALF, d_]
                blocks.append(blk)
    bmats = np.concatenate(blocks, axis=1)                    # [9, 1024]
    return np.ascontiguousarray(bmats.astype(ml_dtypes.bfloat16))


# ------------------------------------------------------------------- kernel
def kernel(curve, noise, speeds_table, braking_limits_table, deltaT):
    curve = np.asarray(curve, np.float32)
    noise = np.asarray(noise, np.float32)
    xp = np.asarray(speeds_table, np.float32)
    fp = np.asarray(braking_limits_table, np.float32)
    dT = float(np.asarray(deltaT))

    A1, A2 = _coeff_matrices(dT)                    # [256, 8] each
    c1 = A1 @ curve                                 # [256, 2]
    c2 = A2 @ curve

    lin_ab = _interp_params(xp, fp)
    if lin_ab is not None:
        a, b = lin_ab
        generic = None
    else:
        xpd = xp.astype(np.float64)
        fpd = fp.astype(np.float64)
        slopes = np.diff(fpd) / np.diff(xpd)
        d = np.concatenate([[slopes[0]], np.diff(slopes)])
        generic = (list(map(float, xpd[:-1])), list(map(float, d)), float(fpd[0]))
        a, b = 0.0, 0.0
    xmin, xmax = float(xp[0]), float(xp[-1])

    bmats = _build_bmats(A1, A2, c1, c2)
    prog = _get_program((a, b, xmin, xmax), generic)
    in_maps = _core_inputs(noise, bmats)

    from concourse.bass_utils import run_bass_kernel_spmd
    res = run_bass_kernel_spmd(prog, in_maps, list(range(N_CORES)))
    global LAST_RESULTS
    LAST_RESULTS = res
    red = np.concatenate([res.results[i]["red"].reshape(-1)
                          for i in range(N_CORES)])

    spd = np.exp(-BETA_BRAKE / NUM_POINTS * red.astype(np.float64))
    probs = spd / spd.sum()
    wsum = probs @ noise.reshape(NUM_SAMPLES, -1).astype(np.float64)
    out = curve.astype(np.float64) + wsum.reshape(ORDER + 1, 2)
    return out.astype(np.float32)


# revision 38
# speedup vs baseline: 1.0591x; 1.0581x over previous
"""Trainium2 Bass kernel for the BayesianFilter (racing-line posterior) problem.

Math (per sample s, P=256 curve points, n=7 Bezier order):
    curves = curve + noise[s]                       # [8,2]
    v  = (M_D1 @ (n*D1) @ curves) / dT              # [P,2]
    a  = (M_D2 @ (n*(n-1)*D2) @ curves) / dT^2      # [P,2]
    speed = |v|, lin = (a.v)/speed
    blim = interp(speed, xp, fp)   (piecewise linear, clamped)
    viol = min(lin - blim, 0);  brake = exp(mean_p viol)
    ca_score = clip(exp(relu(...)), 0, 1) == 1.0 identically  -> dropped
    sp = brake;  out = sum_s (sp/sum sp) * curves[s]

Device computes red[s] = sum_p relu(blim - lin) for all samples
(data-parallel over 8 cores, 8192 samples each); the exp, normalization and
the tiny weighted [8,2] sum run on host.

Device layout: partitions = 128 curve points (2 halves), free = samples.
    v/a via PE matmuls (bf16 in, f32 PSUM out):
        out[p, s] = B[9,128].T @ noise_aug[9, s]
    (noise_aug rows = 8 transposed noise components + ones row; B rows =
    folded coefficient matrix + bias column from `curve`).

Engine budget per half-block (x|y pairs packed in [128,1024] PSUM tiles;
cost-model busy per core: ACT 98us, DVE 95us, POOL 72us, PE 35us, end 122us):
    PE  : 4 matmuls (bf16) into vxy/axy pairs + 2 column-sum matmuls
    ACT : copy(axy) [PSUM->SBUF bf16], square(vxy) -> bf16, sqrt
          (one activation table set -> a single table load)
    DVE : dprod = caxy*vxy (1x, PSUM operand), s2 add (bf16 2x),
          recip_approx(speed), bclip = b*min(speed,xmax) (2x),
          u = bclip-lin (bf16 2x), relu+bias (bf16 4x)
    POOL: dot = dx+dy, lin = dot*rs
"""

import numpy as np
import ml_dtypes
from math import comb

# ---------------------------------------------------------------- constants
NUM_POINTS = 256
ORDER = 7
NUM_SAMPLES = 65536
N_CORES = 8
BETA_BRAKE = 1.0
S_CORE = NUM_SAMPLES // N_CORES          # 8192 samples per core
NBLK = 16                                # sample blocks per core
BLK = S_CORE // NBLK                     # 512 samples per block
HALF = 128                               # points per partition-tile

_PROGRAM_CACHE: dict = {}
LAST_RESULTS = None


def _bezier_matrix(num_points, order):
    s = np.linspace(0.0, 1.0, num_points)[:, None]
    k = np.arange(order + 1)[None, :]
    binom = np.array([comb(order, i) for i in range(order + 1)], dtype=np.float64)[None, :]
    return (binom * (s ** k) * ((1.0 - s) ** (order - k))).astype(np.float32)


def _coeff_matrices(deltaT):
    """A1 [256,8] and A2 [256,8]: point-velocity / acceleration as linear maps
    of the 8 control points (per spatial dim)."""
    n = ORDER
    M1 = _bezier_matrix(NUM_POINTS, n - 1).astype(np.float64)   # [P, 7]
    M2 = _bezier_matrix(NUM_POINTS, n - 2).astype(np.float64)   # [P, 6]
    D1 = np.zeros((n, n + 1))
    for j in range(n):
        D1[j, j] = -1.0
        D1[j, j + 1] = 1.0
    D2 = np.zeros((n - 1, n + 1))
    for j in range(n - 1):
        D2[j, j] = 1.0
        D2[j, j + 1] = -2.0
        D2[j, j + 2] = 1.0
    A1 = (M1 @ (n * D1)) / float(deltaT)
    A2 = (M2 @ (n * (n - 1) * D2)) / (float(deltaT) ** 2)
    return A1.astype(np.float32), A2.astype(np.float32)


def _interp_params(xp, fp):
    """If the table is a strictly-increasing, globally-linear ramp return
    (a, b) with f(x) = a + b*clip(x, xp[0], xp[-1]); else None."""
    xp = np.asarray(xp, np.float64)
    fp = np.asarray(fp, np.float64)
    dx = np.diff(xp)
    if not (dx > 0).all():
        return None
    slopes = np.diff(fp) / dx
    b = slopes[0]
    if not np.allclose(slopes, b, rtol=1e-5, atol=1e-7):
        return None
    a = fp[0] - b * xp[0]
    return float(a), float(b)


# ------------------------------------------------------------ device program
def _build_program(a, b, xmin, xmax, generic_knots=None):
    """Trace + compile the single-core SPMD program.

    Inputs (per core): bmats [9, 1024] bf16, nx [9, 8192] bf16, ny [9, 8192] bf16.
    Output: red [16, 512] f32 — per-sample sum_p relu(blim - lin)
    (host computes sp = exp(-BETA/P * red)).

    generic_knots: None for the linear-interp fast path, else a tuple
    (xp list[16], d list[15], y0) for the relu-sum piecewise path.
    """
    import concourse.bacc as bacc
    import concourse.tile as tile
    import concourse.mybir as mybir

    f32 = mybir.dt.float32
    bf16 = mybir.dt.bfloat16
    Act = mybir.ActivationFunctionType
    Alu = mybir.AluOpType

    nc = bacc.Bacc("TRN2", target_bir_lowering=False, debug=False)

    bmats_d = nc.dram_tensor("bmats", [9, 8 * HALF], bf16, kind="ExternalInput").ap()
    nx_d = nc.dram_tensor("nx", [9, S_CORE], bf16, kind="ExternalInput").ap()
    ny_d = nc.dram_tensor("ny", [9, S_CORE], bf16, kind="ExternalInput").ap()
    # NOTE: 1-D ExternalOutput tensors fail at NEFF LoadExecutable under the
    # axon/PJRT path — keep DRAM I/O 2-D.
    red_d = nc.dram_tensor("red", [NBLK, BLK], f32, kind="ExternalOutput").ap()

    with tile.TileContext(nc) as tc:
        with (
            tc.tile_pool(name="const", bufs=1) as const_pool,
            tc.tile_pool(name="rhs", bufs=4) as rhs_pool,
            tc.tile_pool(name="work", bufs=6) as work,
            tc.tile_pool(name="spout", bufs=4) as spout_pool,
            tc.tile_pool(name="mmv", bufs=2, space="PSUM") as mmv_pool,
            tc.tile_pool(name="mma", bufs=1, space="PSUM") as mma_pool,
            tc.tile_pool(name="red", bufs=1, space="PSUM") as red_pool,
        ):
            bm = const_pool.tile([9, 8 * HALF], bf16, tag="bm")
            nc.sync.dma_start(bm[:], bmats_d)
            ones = const_pool.tile([HALF, 1], bf16, tag="ones")
            nc.gpsimd.memset(ones[:], 1.0)
            # pre-warm the sqrt-set activation table while input DMAs run
            warm = const_pool.tile([HALF, 1], f32, tag="warm")
            nc.gpsimd.memset(warm[:], 1.0)
            warm2 = const_pool.tile([HALF, 1], f32, tag="warm2")
            nc.scalar.sqrt(warm2[:], warm[:])
            bias_knots = []
            if generic_knots is not None:
                for i, xk in enumerate(generic_knots[0]):
                    t = const_pool.tile([HALF, 1], f32, tag=f"bias_k{i}")
                    nc.vector.memset(t[:], -float(xk))
                    bias_knots.append(t)

            # lhsT blocks in bmats: [vx_h0, vx_h1, vy_h0, vy_h1, ax_h0, ax_h1, ay_h0, ay_h1]
            def bmat(i):
                return bm[:, i * HALF:(i + 1) * HALF]

            pending_out = []

            def flush_out():
                while pending_out:
                    kk, t = pending_out.pop(0)
                    nc.sync.dma_start(red_d[kk:kk + 1, :], t[:])

            rxp = ryp = None
            for k in range(NBLK):
                if k % 2 == 0:
                    rxp = rhs_pool.tile([9, 2 * BLK], bf16, tag="rx")
                    nc.sync.dma_start(rxp[:], nx_d[:, k * BLK:(k + 2) * BLK])
                    ryp = rhs_pool.tile([9, 2 * BLK], bf16, tag="ry")
                    nc.sync.dma_start(ryp[:], ny_d[:, k * BLK:(k + 2) * BLK])
                ks = slice((k % 2) * BLK, (k % 2 + 1) * BLK)
                rx = rxp[:, ks]
                ry = ryp[:, ks]
                flush_out()

                red = red_pool.tile([1, BLK], f32, tag="red")
                # SBUF tail paired across halves: h0 -> [:, :BLK], h1 -> [:, BLK:]
                s2p = work.tile([HALF, 2 * BLK], bf16, tag="s2p")
                dotp = work.tile([HALF, 2 * BLK], bf16, tag="dotp")
                for h in range(2):
                    hs = slice(h * BLK, (h + 1) * BLK)
                    # x|y paired PSUM tiles: one 2-bank tile per (v, a)
                    vxy = mmv_pool.tile([HALF, 2 * BLK], f32, tag="vxy")
                    nc.tensor.matmul(vxy[:, 0:BLK], bmat(0 + h), rx[:],
                                     start=True, stop=True)
                    nc.tensor.matmul(vxy[:, BLK:2 * BLK], bmat(2 + h), ry[:],
                                     start=True, stop=True)
                    axy = mma_pool.tile([HALF, 2 * BLK], f32, tag="axy")
                    nc.tensor.matmul(axy[:, 0:BLK], bmat(4 + h), rx[:],
                                     start=True, stop=True)
                    nc.tensor.matmul(axy[:, BLK:2 * BLK], bmat(6 + h), ry[:],
                                     start=True, stop=True)

                    # ACT: one copy + one square over the x|y pair
                    caxy = work.tile([HALF, 2 * BLK], bf16, tag="caxy")
                    nc.scalar.copy(caxy[:], axy[:])
                    sqp = work.tile([HALF, 2 * BLK], bf16, tag="sqp")
                    nc.scalar.square(sqp[:], vxy[:])
                    # DVE: dprod = a*v for x|y in one pass (PSUM operand, 1x)
                    dprod = work.tile([HALF, 2 * BLK], bf16, tag="dprod")
                    nc.vector.tensor_mul(dprod[:], caxy[:], vxy[:])
                    # DVE bf16 2x: s2 = vx^2 + vy^2 into the paired tile
                    nc.vector.tensor_add(s2p[:, hs], sqp[:, 0:BLK],
                                         sqp[:, BLK:2 * BLK])
                    # POOL: dot = dx + dy into the paired tile
                    nc.gpsimd.tensor_add(dotp[:, hs], dprod[:, 0:BLK],
                                         dprod[:, BLK:2 * BLK])

                # block-wide tail at [128, 2*BLK]
                speed = work.tile([HALF, 2 * BLK], f32, tag="speed")
                nc.scalar.sqrt(speed[:], s2p[:])
                rs = work.tile([HALF, 2 * BLK], f32, tag="rs")
                nc.vector.reciprocal_approx_fast(out=rs[:], in_=speed[:])
                lin = work.tile([HALF, 2 * BLK], bf16, tag="lin")
                nc.gpsimd.tensor_mul(lin[:], dotp[:], rs[:])

                ru = work.tile([HALF, 2 * BLK], bf16, tag="ru")
                if generic_knots is None and xmin <= 0.0:
                    bclip = work.tile([HALF, 2 * BLK], bf16, tag="bclip")
                    nc.vector.tensor_scalar(
                        out=bclip[:], in0=speed[:],
                        scalar1=float(xmax), scalar2=float(b),
                        op0=Alu.min, op1=Alu.mult,
                    )
                    u = work.tile([HALF, 2 * BLK], bf16, tag="u")
                    nc.vector.tensor_sub(u[:], bclip[:], lin[:])
                    nc.vector.tensor_scalar(
                        out=ru[:], in0=u[:],
                        scalar1=float(a), scalar2=0.0,
                        op0=Alu.add, op1=Alu.max,
                    )
                elif generic_knots is None:
                    clipv = work.tile([HALF, 2 * BLK], bf16, tag="clipv")
                    nc.vector.tensor_scalar(
                        out=clipv[:], in0=speed[:],
                        scalar1=float(xmin), scalar2=float(xmax),
                        op0=Alu.max, op1=Alu.min,
                    )
                    u = work.tile([HALF, 2 * BLK], bf16, tag="u")
                    nc.vector.scalar_tensor_tensor(
                        out=u[:], in0=clipv[:], scalar=float(b), in1=lin[:],
                        op0=Alu.mult, op1=Alu.subtract,
                    )
                    nc.vector.tensor_scalar(
                        out=ru[:], in0=u[:],
                        scalar1=float(a), scalar2=0.0,
                        op0=Alu.add, op1=Alu.max,
                    )
                else:
                    xp_k, d_k, y0 = generic_knots
                    clipv = work.tile([HALF, 2 * BLK], f32, tag="clipv")
                    nc.vector.tensor_scalar(
                        out=clipv[:], in0=speed[:],
                        scalar1=float(xp_k[0]), scalar2=float(xp_k[-1]),
                        op0=Alu.max, op1=Alu.min,
                    )
                    # blim(x) = y0 + sum_i d_i * relu(x - xp_i)
                    acc = work.tile([HALF, 2 * BLK], f32, tag="acc")
                    ri = work.tile([HALF, 2 * BLK], f32, tag="ri")
                    nc.scalar.activation(ri[:], clipv[:], Act.Relu,
                                         bias=bias_knots[0][:])
                    nc.vector.tensor_scalar(
                        out=acc[:], in0=ri[:],
                        scalar1=float(d_k[0]), scalar2=float(y0),
                        op0=Alu.mult, op1=Alu.add,
                    )
                    for i in range(1, len(d_k)):
                        ri = work.tile([HALF, 2 * BLK], f32, tag="ri")
                        nc.scalar.activation(ri[:], clipv[:], Act.Relu,
                                             bias=bias_knots[i][:])
                        nc.vector.scalar_tensor_tensor(
                            out=acc[:], in0=ri[:], scalar=float(d_k[i]),
                            in1=acc[:], op0=Alu.mult, op1=Alu.add,
                        )
                    u = work.tile([HALF, 2 * BLK], f32, tag="u")
                    nc.vector.tensor_sub(u[:], acc[:], lin[:])
                    nc.vector.tensor_scalar(
                        out=ru[:], in0=u[:], scalar1=0.0, scalar2=None,
                        op0=Alu.max,
                    )

                # red[0, s] += sum_p ru[p, s]   (PE column-sum, bf16 in f32 acc)
                nc.tensor.matmul(red[:], ones[:], ru[:, 0:BLK],
                                 start=True, stop=False)
                nc.tensor.matmul(red[:], ones[:], ru[:, BLK:2 * BLK],
                                 start=False, stop=True)

                out_t = spout_pool.tile([1, BLK], f32, tag="out")
                nc.scalar.copy(out_t[:], red[:])
                pending_out.append((k, out_t))
            flush_out()

    nc.compile()
    return nc


def _get_program(key_params, generic_knots=None):
    key = (key_params, None if generic_knots is None else
           (tuple(generic_knots[0]), tuple(generic_knots[1]), generic_knots[2]))
    prog = _PROGRAM_CACHE.get(key)
    if prog is None:
        a, b, xmin, xmax = key_params
        prog = _build_program(a, b, xmin, xmax, generic_knots)
        _PROGRAM_CACHE[key] = prog
    return prog


def _core_inputs(noise, bmats_bf):
    """Per-core input dicts: transposed bf16 noise components + ones row."""
    ins = []
    for cidx in range(N_CORES):
        sl = noise[cidx * S_CORE:(cidx + 1) * S_CORE]        # [8192, 8, 2]
        nxa = np.empty((9, S_CORE), ml_dtypes.bfloat16)
        nxa[:8] = sl[:, :, 0].T.astype(ml_dtypes.bfloat16)
        nxa[8] = 1.0
        nya = np.empty((9, S_CORE), ml_dtypes.bfloat16)
        nya[:8] = sl[:, :, 1].T.astype(ml_dtypes.bfloat16)
        nya[8] = 1.0
        ins.append({"bmats": bmats_bf, "nx": np.ascontiguousarray(nxa),
                    "ny": np.ascontiguousarray(nya)})
    return ins


def _build_bmats(A1, A2, c1, c2):
    # bmats: 8 blocks [9, 128]: rows 0-7 = A.T half, row 8 = bias column
    # order: vx_h0, vx_h1, vy_h0, vy_h1, ax_h0, ax_h1, ay_h0, ay_h1
    blocks = []
    for (A, c) in ((A1, c1), (A2, c2)):
        for d_ in range(2):
            for h in range(2):
                blk = np.empty((9, HALF), np.float32)
                blk[:8] = A[h * HALF:(h + 1) * HALF, :].T
                blk[8] = c[h * HALF:(h + 1) * HALF, d_]
                blocks.append(blk)
    bmats = np.concatenate(blocks, axis=1)                    # [9, 1024]
    return np.ascontiguousarray(bmats.astype(ml_dtypes.bfloat16))


# ------------------------------------------------------------------- kernel
def kernel(curve, noise, speeds_table, braking_limits_table, deltaT):
    curve = np.asarray(curve, np.float32)
    noise = np.asarray(noise, np.float32)
    xp = np.asarray(speeds_table, np.float32)
    fp = np.asarray(braking_limits_table, np.float32)
    dT = float(np.asarray(deltaT))

    A1, A2 = _coeff_matrices(dT)                    # [256, 8] each
    c1 = A1 @ curve                                 # [256, 2]
    c2 = A2 @ curve

    lin_ab = _interp_params(xp, fp)
    if lin_ab is not None:
        a, b = lin_ab
        generic = None
    else:
        xpd = xp.astype(np.float64)
        fpd = fp.astype(np.float64)
        slopes = np.diff(fpd) / np.diff(xpd)
        d = np.concatenate([[slopes[0]], np.diff(slopes)])
        generic = (list(map(float, xpd[:-1])), list(map(float, d)), float(fpd[0]))
        a, b = 0.0, 0.0
    xmin, xmax = float(xp[0]), float(xp[-1])

    bmats = _build_bmats(A1, A2, c1, c2)
    prog = _get_program((a, b, xmin, xmax), generic)
    in_maps = _core_inputs(noise, bmats)

    from concourse.bass_utils import run_bass_kernel_spmd
    res = run_bass_kernel_spmd(prog, in_maps, list(range(N_CORES)))
    global LAST_RESULTS
    LAST_RESULTS = res
    red = np.concatenate([res.results[i]["red"].reshape(-1)
                          for i in range(N_CORES)])

    spd = np.exp(-BETA_BRAKE / NUM_POINTS * red.astype(np.float64))
    probs = spd / spd.sum()
    wsum = probs @ noise.reshape(NUM_SAMPLES, -1).astype(np.float64)
    out = curve.astype(np.float64) + wsum.reshape(ORDER + 1, 2)
    return out.astype(np.float32)
